# revision 11
# baseline (speedup 1.0000x reference)
"""Multi-head causal self-attention (B=2, S=2048, D=2048, H=16) on 8 TRN2 cores.

Sharding: data parallel on batch (2) x tensor parallel on head groups (4 heads
per core). Each core computes QKV projections for its 512 q/k/v channels, the
causal attention for its 4 heads, and a partial output projection against its
512 columns of Wo. The host sums the 4 partials per batch and adds bo.

Schedule (v3): one dense in-order PE stream.
 - consts ride the scalar HWDGE queue; sync starts with the first matmul's
   xt/weight tiles. wo prefetch is issued at the attention boundary.
 - qk drains split vector/scalar/gpsimd in next-use order.
 - attention: scores are computed in [k, q] orientation into 2-bank
   [128, 1024] psum pairs, exp'd in one activation per pair (bias -ln4 keeps
   the fp8 row-sum operand in range; the /4 cancels through the reciprocal).
 - row sums: fp8 DoubleRow matmul per full pair (half rate), fp16 ones
   matmul for the 4 masked diagonal tiles of each unit.
 - a lag-1 pair software pipeline runs across unit boundaries; oproj units
   of the previous g are spliced between pairs as ready PE filler.
 - output partials are written fp16 on the sync HWDGE queue.
"""

import math
from contextlib import ExitStack

import numpy as np

import concourse.bass as bass
import concourse.tile as tile
from concourse import bacc, mybir
from concourse.bass_utils import run_bass_kernel_spmd

B, S, D, H, HD = 2, 2048, 2048, 16, 128
N_CORES = 8
HPC = 4          # heads per core
HJ = HPC * HD    # 512 projection channels per core
SG = 512         # column-group width for matmuls
ND = D // 128    # 16 contraction tiles over model dim
NS = S // 128    # 16 tiles over sequence
NG = S // SG     # 4 column groups over sequence

F32 = mybir.dt.float32
F16 = mybir.dt.float16
F8 = mybir.dt.float8e4
AX = mybir.AxisListType.X
ADD = mybir.AluOpType.add
MUL = mybir.AluOpType.mult
EXP = mybir.ActivationFunctionType.Exp
IDENT = mybir.ActivationFunctionType.Identity
DR = mybir.MatmulPerfMode.DoubleRow

LN4 = math.log(4.0)
USE_FP8_SUM = False

last_exec_time_ns = None


def _build():
    nc = bacc.Bacc("TRN2", target_bir_lowering=False, debug=False)

    xt = nc.dram_tensor("xt", [D, S], F16, kind="ExternalInput").ap()
    wq = nc.dram_tensor("wq", [D, HJ], F16, kind="ExternalInput").ap()
    wk = nc.dram_tensor("wk", [D, HJ], F16, kind="ExternalInput").ap()
    wv = nc.dram_tensor("wv", [D, HJ], F16, kind="ExternalInput").ap()
    wo = nc.dram_tensor("wo", [HJ, D], F16, kind="ExternalInput").ap()
    bq = nc.dram_tensor("bq", [HJ, 1], F32, kind="ExternalInput").ap()
    bk = nc.dram_tensor("bk", [HJ, 1], F32, kind="ExternalInput").ap()
    bv = nc.dram_tensor("bv", [1, HJ], F16, kind="ExternalInput").ap()
    mask = nc.dram_tensor("mask", [128, 128], F32, kind="ExternalInput").ap()
    out = nc.dram_tensor("out", [S, D], F16, kind="ExternalOutput").ap()

    with tile.TileContext(nc) as tc, ExitStack() as es:
        cpool = es.enter_context(tc.tile_pool(name="const", bufs=1))
        mask_sb = cpool.tile([128, 128], F32, name="mask", tag="mask")
        nc.scalar.dma_start(mask_sb[:], mask[:])
        bv_sb = cpool.tile([1, HJ], F16, name="bv", tag="bv")
        nc.scalar.dma_start(bv_sb[:], bv[:])
        onesm_sb = cpool.tile([128, 128], F16, name="onesm_sb", tag="onesm")
        nc.gpsimd.memset(onesm_sb[:], 1.0)
        ones8_sb = cpool.tile([128, 2, 128], F8, name="ones8_sb", tag="ones8")
        nc.gpsimd.memset(ones8_sb[:], 1.0)
        negln4_sb = cpool.tile([128, 1], F32, name="negln4_sb", tag="negln4")
        nc.gpsimd.memset(negln4_sb[:], -LN4)
        bq_sb = []
        bk_sb = []
        for i in range(HPC):
            t = cpool.tile([128, 1], F32, name=f"bq{i}", tag=f"bq{i}")
            nc.scalar.dma_start(t[:], bq[i * 128:(i + 1) * 128, :])
            bq_sb.append(t)
            t = cpool.tile([128, 1], F32, name=f"bk{i}", tag=f"bk{i}")
            nc.scalar.dma_start(t[:], bk[i * 128:(i + 1) * 128, :])
            bk_sb.append(t)

        rpool = es.enter_context(tc.tile_pool(name="res", bufs=1))
        qT = [rpool.tile([128, S], F16, name=f"qT{i}", tag=f"qT{i}")
              for i in range(HPC)]
        kT = [rpool.tile([128, S], F16, name=f"kT{i}", tag=f"kT{i}")
              for i in range(HPC)]
        vsb = [rpool.tile([128, HJ], F16, name=f"v{j}", tag=f"v{j}")
               for j in range(NS)]
        attn = [rpool.tile([128, S], F16, name=f"at{h}", tag=f"at{h}")
                for h in range(HPC)]
        wot = [rpool.tile([128, D], F16, name=f"wo{t_}", tag=f"wo{t_}")
               for t_ in range(HPC)]

        # ---------------- phase 1: q/k/v projections ----------------------
        with tc.tile_pool(name="wqk", bufs=1) as wpool, \
             tc.tile_pool(name="xt1", bufs=8) as xpool, \
             tc.tile_pool(name="wvp", bufs=1) as wvpool, \
             tc.tile_pool(name="xtv", bufs=8) as vxpool, \
             tc.tile_pool(name="ps1", bufs=1, space="PSUM") as ps1:
            wtile = {}
            wvt = {}
            for sg in range(NG):
                if sg == NG - 2:
                    for d in range(ND):
                        t = wvpool.tile([128, HJ], F16, name=f"wv{d}",
                                        tag=f"wv{d}")
                        nc.sync.dma_start(t[:], wv[d * 128:(d + 1) * 128, :])
                        wvt[d] = t
                ps = {}
                for i in range(HPC):
                    ps[("q", i)] = ps1.tile([128, SG], F32, name=f"psa{i}",
                                            tag=f"a{i}")
                    ps[("k", i)] = ps1.tile([128, SG], F32, name=f"psb{i}",
                                            tag=f"b{i}")
                for d in range(ND):
                    xtile = xpool.tile([128, SG], F16, name="xtile", tag="xt")
                    nc.sync.dma_start(
                        xtile[:], xt[d * 128:(d + 1) * 128,
                                     sg * SG:(sg + 1) * SG])
                    for which, wdram in (("q", wq), ("k", wk)):
                        if (which, d) not in wtile:
                            t = wpool.tile([128, SG], F16, name=f"w{which}{d}",
                                           tag=f"w{which}{d}")
                            nc.sync.dma_start(
                                t[:], wdram[d * 128:(d + 1) * 128, :])
                            wtile[(which, d)] = t
                        # emission order q_i, k_i interleaved matches the
                        # 3-way drain completion order below
                        for i in range(HPC):
                            nc.tensor.matmul(
                                ps[(which, i)][:],
                                lhsT=wtile[(which, d)][:, i * 128:(i + 1) * 128],
                                rhs=xtile[:],
                                start=(d == 0), stop=(d == ND - 1))
                # drains split vector/scalar in next-use order
                for i in range(HPC):
                    nc.vector.tensor_scalar_add(
                        qT[i][:, sg * SG:(sg + 1) * SG], ps[("q", i)][:],
                        bq_sb[i][:])
                    nc.scalar.activation(
                        kT[i][:, sg * SG:(sg + 1) * SG], ps[("k", i)][:],
                        IDENT, bias=bk_sb[i][:])

            # v pass (re-streams x^T; psum banks reuse the q/k tags,
            # alternating by sg parity for cross-sg overlap)
            for sg in range(NG):
                ab = "a" if sg % 2 == 0 else "b"
                ps = [ps1.tile([128, HJ], F32, name=f"psv{i}", tag=f"{ab}{i}")
                      for i in range(4)]
                for d in range(ND):
                    xtile = vxpool.tile([128, SG], F16, name="xtile", tag="xt")
                    nc.sync.dma_start(
                        xtile[:], xt[d * 128:(d + 1) * 128,
                                     sg * SG:(sg + 1) * SG])
                    for ss in range(4):
                        nc.tensor.matmul(
                            ps[ss][:],
                            lhsT=xtile[:, ss * 128:(ss + 1) * 128],
                            rhs=wvt[d][:],
                            start=(d == 0), stop=False)
                for ss in range(4):
                    nc.tensor.matmul(
                        ps[ss][:], lhsT=onesm_sb[0:1, :],
                        rhs=bv_sb[:], start=False, stop=True)
                    if ss % 2 == 0:
                        nc.vector.tensor_copy(vsb[sg * 4 + ss][:], ps[ss][:])
                    else:
                        nc.scalar.copy(vsb[sg * 4 + ss][:], ps[ss][:])

        # wo prefetch for the attention/oproj phase (scalar HWDGE; queued
        # behind the phase-1 drains so it can't crowd early xt tiles)
        for t_ in range(HPC):
            nc.scalar.dma_start(wot[t_][:], wo[t_ * 128:(t_ + 1) * 128, :])

        # ---------------- phases 2+3: attention + output projection -------
        with tc.tile_pool(name="et", bufs=5) as etpool, \
             tc.tile_pool(name="et8", bufs=4) as et8pool, \
             tc.tile_pool(name="sm", bufs=6) as spool, \
             tc.tile_pool(name="ost", bufs=4) as opool, \
             tc.tile_pool(name="ps_sc", bufs=2, space="PSUM") as ps_sc, \
             tc.tile_pool(name="ps_x", bufs=1, space="PSUM") as ps_x, \
             tc.tile_pool(name="ps_pv", bufs=2, space="PSUM") as ps_pv, \
             tc.tile_pool(name="ps_o", bufs=1, space="PSUM") as ps_o:

            ocnt = [0]

            def oproj_unit(st, dg, pool):
                ptag = {id(ps_o): "opj", id(ps_pv): "pv", id(ps_x): "x"}
                po3 = pool.tile([128, SG], F32, name="po3",
                                tag=ptag[id(pool)])
                for h in range(HPC):
                    nc.tensor.matmul(
                        po3[:],
                        lhsT=attn[h][:, st * 128:(st + 1) * 128],
                        rhs=wot[h][:, dg * SG:(dg + 1) * SG],
                        start=(h == 0), stop=(h == HPC - 1))
                ot = opool.tile([128, SG], F16, name="ost", tag="ost")
                if ocnt[0] % 2 == 0:
                    nc.vector.tensor_copy(ot[:], po3[:])
                else:
                    nc.scalar.copy(ot[:], po3[:])
                ocnt[0] += 1
                nc.sync.dma_start(
                    out[st * 128:(st + 1) * 128,
                        dg * SG:(dg + 1) * SG], ot[:])

            # ---- lag-1 pair pipeline across all (g, h) units -------------
            # each unit (g, h): npair = 2g + 2 pairs of k-tiles. Pairs with
            # kt < 4g are "full" (width 512 each); the last two pairs hold
            # the 4 masked diagonal tiles (widths 512/384 and 256/128).
            pend = []        # emitted scores awaiting pv/sum
            state = {}       # per-unit psum tiles

            def emit_scores(g, h, p):
                npair = 2 * g + 2
                if p == 0:
                    state[(g, h)] = (
                        ps_pv.tile([128, SG], F32, name="popv", tag="pv"),
                        ps_x.tile([128, SG], F32, name="smps", tag="x"),
                    )
                psc = ps_sc.tile([128, 2, SG], F32, name="psc", tag="sc")
                et = etpool.tile([128, 2, SG], F16, name="et", tag="et")
                full = (p < 2 * g)
                lo = SG
                for side in range(2):
                    kt = 2 * p + side
                    jlo = max(0, kt - 4 * g)
                    qoff = jlo * 128
                    lo = min(lo, qoff)
                    nc.tensor.matmul(
                        psc[:, side, qoff:],
                        lhsT=kT[h][:, kt * 128:(kt + 1) * 128],
                        rhs=qT[h][:, g * SG + qoff:(g + 1) * SG],
                        start=True, stop=True)
                    if kt >= 4 * g:
                        nc.vector.tensor_tensor(
                            psc[:, side, qoff:qoff + 128],
                            psc[:, side, qoff:qoff + 128],
                            mask_sb[:], op=ADD)
                # one exp per pair; -ln4 scales exp output by 1/4 (fp8 range
                # headroom for the row-sum; cancels through the reciprocal)
                nc.scalar.activation(et[:, :, lo:], psc[:, :, lo:], EXP,
                                     bias=negln4_sb[:])
                et8 = None
                if full and USE_FP8_SUM:
                    et8 = et8pool.tile([128, 2, SG], F8, name="et8", tag="et8")
                    nc.gpsimd.dma_start(et8[:], et[:])
                pend.append((g, h, p, et, et8))

            def emit_pvsum(g, h, p, et, et8):
                po, sm = state[(g, h)]
                npair = 2 * g + 2
                nkt = 4 * g + 4
                for side in range(2):
                    kt = 2 * p + side
                    jlo = max(0, kt - 4 * g)
                    qoff = jlo * 128
                    nc.tensor.matmul(
                        po[:, qoff:],
                        lhsT=vsb[kt][:, h * 128:(h + 1) * 128],
                        rhs=et[:, side, qoff:],
                        start=(kt == 0), stop=(kt == nkt - 1))
                if et8 is not None:
                    nc.tensor.matmul(
                        sm[:], lhsT=ones8_sb[:], rhs=et8[:],
                        start=(p == 0), stop=False,
                        perf_mode=DR, skip_group_check=True)
                else:
                    for side in range(2):
                        kt = 2 * p + side
                        jlo = max(0, kt - 4 * g)
                        qoff = jlo * 128
                        nc.tensor.matmul(
                            sm[:, qoff:],
                            lhsT=onesm_sb[:],
                            rhs=et[:, side, qoff:],
                            start=(kt == 0), stop=(kt == nkt - 1),
                            skip_group_check=True)
                if p == npair - 1:
                    rr = spool.tile([128, SG], F32, name="rr", tag="rr")
                    nc.vector.reciprocal(rr[:], sm[:])
                    nc.vector.tensor_tensor(
                        attn[h][:, g * SG:(g + 1) * SG], po[:], rr[:], op=MUL)
                    del state[(g, h)]

            for g in range(NG - 1, -1, -1):
                if g == NG - 1:
                    filler = []
                else:
                    gp = g + 1
                    filler = [
                        (st, dg)
                        for st in range(4 * gp + 3, 4 * gp - 1, -1)
                        for dg in range(NG)
                    ]
                npair = 2 * g + 2
                total = HPC * npair
                done = 0
                for h in range(HPC):
                    for p in range(npair):
                        emit_scores(g, h, p)
                        if len(pend) > 1:
                            emit_pvsum(*pend.pop(0))
                        done += 1
                        # spread the 16 filler oproj units over the g's pairs
                        while filler and \
                                len(filler) > (total - done) * 16 // total:
                            st, dg = filler.pop(0)
                            oproj_unit(st, dg, ps_o)
                while len(pend) > 1:
                    emit_pvsum(*pend.pop(0))
                for st_dg in filler:
                    oproj_unit(*st_dg, ps_o)

            emit_pvsum(*pend.pop(0))

            # final oproj block for g=0 (rows 0..3); rotate over the freed
            # pv/x banks plus the opj bank for a 4-deep pipeline
            fin = 0
            for st in range(3, -1, -1):
                for dg in range(NG):
                    pool = (ps_o, ps_pv, ps_pv, ps_x)[fin % 4]
                    oproj_unit(st, dg, pool)
                    fin += 1

    nc.finalize()
    return nc


_NC_CACHE = []


def kernel(hidden_states, Wq, bq, Wk, bk, Wv, bv, Wo, bo, **_unused):
    global last_exec_time_ns

    hidden_states = np.asarray(hidden_states, dtype=np.float32)
    Wq = np.asarray(Wq, dtype=np.float32)
    Wk = np.asarray(Wk, dtype=np.float32)
    Wv = np.asarray(Wv, dtype=np.float32)
    Wo = np.asarray(Wo, dtype=np.float32)
    bq = np.asarray(bq, dtype=np.float32)
    bk = np.asarray(bk, dtype=np.float32)
    bv = np.asarray(bv, dtype=np.float32)
    bo = np.asarray(bo, dtype=np.float32)

    if not _NC_CACHE:
        _NC_CACHE.append(_build())
    nc = _NC_CACHE[0]

    scale = 1.0 / math.sqrt(HD)
    q_idx = np.arange(128)[:, None]
    k_idx = np.arange(128)[None, :]
    # [k, q] orientation: keep k <= q
    mask = np.where(k_idx.T <= q_idx.T, 0.0, -50.0).astype(np.float32)

    xts = [np.ascontiguousarray(hidden_states[b].T).astype(np.float16)
           for b in range(B)]
    in_maps = []
    for c in range(N_CORES):
        b, hg = divmod(c, HPC)
        sl = slice(hg * HJ, (hg + 1) * HJ)
        in_maps.append({
            "xt": xts[b],
            "wq": np.ascontiguousarray((Wq[sl] * scale).T).astype(np.float16),
            "wk": np.ascontiguousarray(Wk[sl].T).astype(np.float16),
            "wv": np.ascontiguousarray(Wv[sl].T).astype(np.float16),
            "wo": np.ascontiguousarray(Wo[:, sl].T).astype(np.float16),
            "bq": (bq[sl] * scale).reshape(HJ, 1).copy(),
            "bk": bk[sl].reshape(HJ, 1).copy(),
            "bv": bv[sl].reshape(1, HJ).astype(np.float16),
            "mask": mask,
        })

    res = run_bass_kernel_spmd(nc, in_maps, core_ids=list(range(N_CORES)))
    last_exec_time_ns = res.exec_time_ns

    outp = np.empty((B, S, D), np.float32)
    for b in range(B):
        acc = res.results[b * HPC]["out"].astype(np.float32)
        for c in range(b * HPC + 1, (b + 1) * HPC):
            acc = acc + res.results[c]["out"].astype(np.float32)
        outp[b] = acc + bo[None, :]
    return outp


# revision 23
# speedup vs baseline: 1.0943x; 1.0943x over previous
"""Multi-head causal self-attention (B=2, S=2048, D=2048, H=16) on 8 TRN2 cores.

Sharding: data parallel on batch (2) x tensor parallel on head groups (4 heads
per core). Each core computes QKV projections for its 512 q/k/v channels, the
causal attention for its 4 heads, and a partial output projection against its
512 columns of Wo. The host sums the 4 partials per batch and adds bo.

Schedule (v3): one dense in-order PE stream.
 - consts ride the scalar HWDGE queue; sync starts with the first matmul's
   xt/weight tiles. wo prefetch is issued at the attention boundary.
 - qk drains split vector/scalar/gpsimd in next-use order.
 - attention: scores are computed in [k, q] orientation into 2-bank
   [128, 1024] psum pairs, exp'd in one activation per pair (bias -ln4 keeps
   the fp8 row-sum operand in range; the /4 cancels through the reciprocal).
 - row sums: fp8 DoubleRow matmul per full pair (half rate), fp16 ones
   matmul for the 4 masked diagonal tiles of each unit.
 - a lag-1 pair software pipeline runs across unit boundaries; oproj units
   of the previous g are spliced between pairs as ready PE filler.
 - output partials are written fp16 on the sync HWDGE queue.
"""

import math
from contextlib import ExitStack

import numpy as np

import concourse.bass as bass
import concourse.tile as tile
from concourse import bacc, mybir
from concourse.bass_utils import run_bass_kernel_spmd

B, S, D, H, HD = 2, 2048, 2048, 16, 128
N_CORES = 8
HPC = 4          # heads per core
HJ = HPC * HD    # 512 projection channels per core
SG = 512         # column-group width for matmuls
ND = D // 128    # 16 contraction tiles over model dim
NS = S // 128    # 16 tiles over sequence
NG = S // SG     # 4 column groups over sequence

F32 = mybir.dt.float32
F16 = mybir.dt.float16
F8 = mybir.dt.float8e4
AX = mybir.AxisListType.X
ADD = mybir.AluOpType.add
MUL = mybir.AluOpType.mult
EXP = mybir.ActivationFunctionType.Exp
LN = mybir.ActivationFunctionType.Ln
IDENT = mybir.ActivationFunctionType.Identity
DR = mybir.MatmulPerfMode.DoubleRow

LN4 = math.log(4.0)
USE_FP8_SUM = False

last_exec_time_ns = None


def _build():
    nc = bacc.Bacc("TRN2", target_bir_lowering=False, debug=False)

    xt = nc.dram_tensor("xt", [D, S], F16, kind="ExternalInput").ap()
    wq = nc.dram_tensor("wq", [D, HJ], F16, kind="ExternalInput").ap()
    wk = nc.dram_tensor("wk", [D, HJ], F16, kind="ExternalInput").ap()
    wv = nc.dram_tensor("wv", [D, HJ], F16, kind="ExternalInput").ap()
    wo = nc.dram_tensor("wo", [HJ, D], F16, kind="ExternalInput").ap()
    bq = nc.dram_tensor("bq", [HJ, 1], F32, kind="ExternalInput").ap()
    bk = nc.dram_tensor("bk", [HJ, 1], F32, kind="ExternalInput").ap()
    bv = nc.dram_tensor("bv", [1, HJ], F16, kind="ExternalInput").ap()
    mask = nc.dram_tensor("mask", [128, 128], F32, kind="ExternalInput").ap()
    out = nc.dram_tensor("out", [S, D], F16, kind="ExternalOutput").ap()

    with tile.TileContext(nc) as tc, ExitStack() as es:
        cpool = es.enter_context(tc.tile_pool(name="const", bufs=1))
        mask_sb = cpool.tile([128, 128], F32, name="mask", tag="mask")
        nc.scalar.dma_start(mask_sb[:], mask[:])
        bv_sb = cpool.tile([1, HJ], F16, name="bv", tag="bv")
        nc.scalar.dma_start(bv_sb[:], bv[:])
        onesm_sb = cpool.tile([128, 128], F16, name="onesm_sb", tag="onesm")
        nc.gpsimd.memset(onesm_sb[:], 1.0)
        ones8_sb = cpool.tile([128, 2, 128], F8, name="ones8_sb", tag="ones8")
        nc.gpsimd.memset(ones8_sb[:], 1.0)
        negln4_sb = cpool.tile([128, 1], F32, name="negln4_sb", tag="negln4")
        nc.gpsimd.memset(negln4_sb[:], -LN4)
        bq_sb = []
        bk_sb = []
        for i in range(HPC):
            t = cpool.tile([128, 1], F32, name=f"bq{i}", tag=f"bq{i}")
            nc.scalar.dma_start(t[:], bq[i * 128:(i + 1) * 128, :])
            bq_sb.append(t)
            t = cpool.tile([128, 1], F32, name=f"bk{i}", tag=f"bk{i}")
            nc.scalar.dma_start(t[:], bk[i * 128:(i + 1) * 128, :])
            bk_sb.append(t)

        rpool = es.enter_context(tc.tile_pool(name="res", bufs=1))
        qT = [rpool.tile([128, S], F16, name=f"qT{i}", tag=f"qT{i}")
              for i in range(HPC)]
        kT = [rpool.tile([128, S], F16, name=f"kT{i}", tag=f"kT{i}")
              for i in range(HPC)]
        vsb = [rpool.tile([128, HJ], F16, name=f"v{j}", tag=f"v{j}")
               for j in range(NS)]
        attn = [rpool.tile([128, S], F16, name=f"at{h}", tag=f"at{h}")
                for h in range(HPC)]
        wot = [rpool.tile([128, D], F16, name=f"wo{t_}", tag=f"wo{t_}")
               for t_ in range(HPC)]

        # ---------------- phase 1: q/k/v projections ----------------------
        with tc.tile_pool(name="wqk", bufs=1) as wpool, \
             tc.tile_pool(name="xt1", bufs=8) as xpool, \
             tc.tile_pool(name="wvp", bufs=1) as wvpool, \
             tc.tile_pool(name="xtv", bufs=8) as vxpool, \
             tc.tile_pool(name="ps1", bufs=1, space="PSUM") as ps1:
            wtile = {}
            wvt = {}
            for sg in range(NG):
                if sg == NG - 2:
                    for d in range(ND):
                        t = wvpool.tile([128, HJ], F16, name=f"wv{d}",
                                        tag=f"wv{d}")
                        nc.sync.dma_start(t[:], wv[d * 128:(d + 1) * 128, :])
                        wvt[d] = t
                ps = {}
                for i in range(HPC):
                    ps[("q", i)] = ps1.tile([128, SG], F32, name=f"psa{i}",
                                            tag=f"a{i}")
                    ps[("k", i)] = ps1.tile([128, SG], F32, name=f"psb{i}",
                                            tag=f"b{i}")
                for d in range(ND):
                    xtile = xpool.tile([128, SG], F16, name="xtile", tag="xt")
                    nc.sync.dma_start(
                        xtile[:], xt[d * 128:(d + 1) * 128,
                                     sg * SG:(sg + 1) * SG])
                    for which, wdram in (("q", wq), ("k", wk)):
                        if (which, d) not in wtile:
                            t = wpool.tile([128, SG], F16, name=f"w{which}{d}",
                                           tag=f"w{which}{d}")
                            nc.sync.dma_start(
                                t[:], wdram[d * 128:(d + 1) * 128, :])
                            wtile[(which, d)] = t
                        # emission order q_i, k_i interleaved matches the
                        # 3-way drain completion order below
                        for i in range(HPC):
                            nc.tensor.matmul(
                                ps[(which, i)][:],
                                lhsT=wtile[(which, d)][:, i * 128:(i + 1) * 128],
                                rhs=xtile[:],
                                start=(d == 0), stop=(d == ND - 1))
                # drains split vector/scalar in next-use order
                for i in range(HPC):
                    nc.vector.tensor_scalar_add(
                        qT[i][:, sg * SG:(sg + 1) * SG], ps[("q", i)][:],
                        bq_sb[i][:])
                    nc.scalar.activation(
                        kT[i][:, sg * SG:(sg + 1) * SG], ps[("k", i)][:],
                        IDENT, bias=bk_sb[i][:])

            # v pass (re-streams x^T; psum banks reuse the q/k tags,
            # alternating by sg parity for cross-sg overlap)
            for sg in range(NG):
                ab = "a" if sg % 2 == 0 else "b"
                ps = [ps1.tile([128, HJ], F32, name=f"psv{i}", tag=f"{ab}{i}")
                      for i in range(4)]
                for d in range(ND):
                    xtile = vxpool.tile([128, SG], F16, name="xtile", tag="xt")
                    nc.sync.dma_start(
                        xtile[:], xt[d * 128:(d + 1) * 128,
                                     sg * SG:(sg + 1) * SG])
                    for ss in range(4):
                        nc.tensor.matmul(
                            ps[ss][:],
                            lhsT=xtile[:, ss * 128:(ss + 1) * 128],
                            rhs=wvt[d][:],
                            start=(d == 0), stop=False)
                for ss in range(4):
                    nc.tensor.matmul(
                        ps[ss][:], lhsT=onesm_sb[0:1, :],
                        rhs=bv_sb[:], start=False, stop=True)
                    if ss % 2 == 0:
                        nc.vector.tensor_copy(vsb[sg * 4 + ss][:], ps[ss][:])
                    else:
                        nc.scalar.copy(vsb[sg * 4 + ss][:], ps[ss][:])

        # wo prefetch for the attention/oproj phase (scalar HWDGE; queued
        # behind the phase-1 drains so it can't crowd early xt tiles)
        for t_ in range(HPC):
            nc.scalar.dma_start(wot[t_][:], wo[t_ * 128:(t_ + 1) * 128, :])

        # ---------------- phases 2+3: attention + output projection -------
        with tc.tile_pool(name="et", bufs=6) as etpool, \
             tc.tile_pool(name="et8", bufs=4) as et8pool, \
             tc.tile_pool(name="sm", bufs=6) as spool, \
             tc.tile_pool(name="ost", bufs=4) as opool, \
             tc.tile_pool(name="ps_sc", bufs=3, space="PSUM") as ps_sc, \
             tc.tile_pool(name="ps_x", bufs=2, space="PSUM") as ps_x, \
             tc.tile_pool(name="ps_pv", bufs=2, space="PSUM") as ps_pv, \
             tc.tile_pool(name="ps_o", bufs=1, space="PSUM") as ps_o:

            ocnt = [0]

            def oproj_unit(st, dg, pool):
                ptag = {id(ps_o): "opj", id(ps_pv): "pv", id(ps_x): "x"}
                po3 = pool.tile([128, SG], F32, name="po3",
                                tag=ptag[id(pool)])
                for h in range(HPC):
                    nc.tensor.matmul(
                        po3[:],
                        lhsT=attn[h][:, st * 128:(st + 1) * 128],
                        rhs=wot[h][:, dg * SG:(dg + 1) * SG],
                        start=(h == 0), stop=(h == HPC - 1))
                ot = opool.tile([128, SG], F16, name="ost", tag="ost")
                if ocnt[0] % 2 == 0:
                    nc.vector.tensor_copy(ot[:], po3[:])
                else:
                    nc.scalar.copy(ot[:], po3[:])
                ocnt[0] += 1
                nc.sync.dma_start(
                    out[st * 128:(st + 1) * 128,
                        dg * SG:(dg + 1) * SG], ot[:])

            # ---- lag-2 tile pipeline across all (g, h) units -------------
            # scores run two k-tiles ahead of pv/sum so the exp latency and
            # the end-of-unit normalization chain never stall the PE queue.
            pend = []        # emitted scores awaiting pv/sum
            state = {}       # per-unit psum tiles
            deferred = []    # end-of-unit normalization chains

            def emit_scores(g, h, kt):
                nkt = 4 * g + 4
                if kt == 0:
                    state[(g, h)] = (
                        ps_pv.tile([128, SG], F32, name="popv", tag="pv"),
                        ps_x.tile([128, SG], F32, name="smps", tag="x"),
                    )
                jlo = max(0, kt - 4 * g)
                qoff = jlo * 128
                psc = ps_sc.tile([128, SG], F32, name="psc", tag="sc")
                et = etpool.tile([128, SG], F16, name="et", tag="et")
                nc.tensor.matmul(
                    psc[:, qoff:],
                    lhsT=kT[h][:, kt * 128:(kt + 1) * 128],
                    rhs=qT[h][:, g * SG + qoff:(g + 1) * SG],
                    start=True, stop=True)
                if kt >= 4 * g:
                    nc.vector.tensor_tensor(
                        psc[:, qoff:qoff + 128], psc[:, qoff:qoff + 128],
                        mask_sb[:], op=ADD)
                # -ln4 scales exp output by 1/4 (fp8 range headroom for the
                # row-sum path; cancels through the normalization)
                nc.scalar.activation(et[:, qoff:], psc[:, qoff:], EXP,
                                     bias=negln4_sb[:])
                pend.append((g, h, kt, et))

            def emit_pvsum(g, h, kt, et):
                po, sm = state[(g, h)]
                nkt = 4 * g + 4
                jlo = max(0, kt - 4 * g)
                qoff = jlo * 128
                nc.tensor.matmul(
                    po[:, qoff:],
                    lhsT=vsb[kt][:, h * 128:(h + 1) * 128],
                    rhs=et[:, qoff:],
                    start=(kt == 0), stop=(kt == nkt - 1))
                nc.tensor.matmul(
                    sm[:, qoff:],
                    lhsT=onesm_sb[:],
                    rhs=et[:, qoff:],
                    start=(kt == 0), stop=(kt == nkt - 1),
                    skip_group_check=True)
                if kt == nkt - 1:
                    # defer the normalization chain a couple of tiles into
                    # the next unit so its first exps get ahead of it in
                    # the in-order queues
                    def _norm(po=po, sm=sm, g=g, h=h):
                        rr = spool.tile([128, SG], F32, name="rr", tag="rr")
                        nc.vector.reciprocal(rr[:], sm[:])
                        nc.vector.tensor_tensor(
                            attn[h][:, g * SG:(g + 1) * SG], po[:], rr[:],
                            op=MUL)
                    deferred.append(_norm)
                    del state[(g, h)]

            for g in range(NG - 1, -1, -1):
                if g == NG - 1:
                    filler = []
                else:
                    gp = g + 1
                    filler = [
                        (st, dg)
                        for st in range(4 * gp + 3, 4 * gp - 1, -1)
                        for dg in range(NG)
                    ]
                nkt = 4 * g + 4
                total = HPC * nkt
                done = 0
                for h in range(HPC):
                    for kt in range(nkt):
                        emit_scores(g, h, kt)
                        if kt == 2 and deferred:
                            deferred.pop(0)()
                        if len(pend) > 2:
                            emit_pvsum(*pend.pop(0))
                        done += 1
                        # spread the 16 filler oproj units over the g's
                        # tiles; done >= 3 guarantees the previous g's last
                        # deferred norm is emitted before the first filler
                        # unit reads that attn block
                        while filler and done >= 3 and \
                                len(filler) > (total - done) * 16 // total:
                            st, dg = filler.pop(0)
                            oproj_unit(st, dg, ps_o)
                while len(pend) > 2:
                    emit_pvsum(*pend.pop(0))
                for st_dg in filler:
                    oproj_unit(*st_dg, ps_o)

            while pend:
                emit_pvsum(*pend.pop(0))
            while deferred:
                deferred.pop(0)()

            # final oproj block for g=0 (rows 0..3); rotate over the freed
            # pv/x banks plus the opj bank for a 4-deep pipeline
            fin = 0
            for st in range(3, -1, -1):
                for dg in range(NG):
                    pool = (ps_o, ps_pv, ps_pv, ps_x)[fin % 4]
                    oproj_unit(st, dg, pool)
                    fin += 1

    nc.finalize()
    return nc


_NC_CACHE = []


def kernel(hidden_states, Wq, bq, Wk, bk, Wv, bv, Wo, bo, **_unused):
    global last_exec_time_ns

    hidden_states = np.asarray(hidden_states, dtype=np.float32)
    Wq = np.asarray(Wq, dtype=np.float32)
    Wk = np.asarray(Wk, dtype=np.float32)
    Wv = np.asarray(Wv, dtype=np.float32)
    Wo = np.asarray(Wo, dtype=np.float32)
    bq = np.asarray(bq, dtype=np.float32)
    bk = np.asarray(bk, dtype=np.float32)
    bv = np.asarray(bv, dtype=np.float32)
    bo = np.asarray(bo, dtype=np.float32)

    if not _NC_CACHE:
        _NC_CACHE.append(_build())
    nc = _NC_CACHE[0]

    scale = 1.0 / math.sqrt(HD)
    q_idx = np.arange(128)[:, None]
    k_idx = np.arange(128)[None, :]
    # [k, q] orientation: keep k <= q
    mask = np.where(k_idx.T <= q_idx.T, 0.0, -50.0).astype(np.float32)

    xts = [np.ascontiguousarray(hidden_states[b].T).astype(np.float16)
           for b in range(B)]
    in_maps = []
    for c in range(N_CORES):
        b, hg = divmod(c, HPC)
        sl = slice(hg * HJ, (hg + 1) * HJ)
        in_maps.append({
            "xt": xts[b],
            "wq": np.ascontiguousarray((Wq[sl] * scale).T).astype(np.float16),
            "wk": np.ascontiguousarray(Wk[sl].T).astype(np.float16),
            "wv": np.ascontiguousarray(Wv[sl].T).astype(np.float16),
            "wo": np.ascontiguousarray(Wo[:, sl].T).astype(np.float16),
            "bq": (bq[sl] * scale).reshape(HJ, 1).copy(),
            "bk": bk[sl].reshape(HJ, 1).copy(),
            "bv": bv[sl].reshape(1, HJ).astype(np.float16),
            "mask": mask,
        })

    res = run_bass_kernel_spmd(nc, in_maps, core_ids=list(range(N_CORES)))
    last_exec_time_ns = res.exec_time_ns

    outp = np.empty((B, S, D), np.float32)
    for b in range(B):
        acc = res.results[b * HPC]["out"].astype(np.float32)
        for c in range(b * HPC + 1, (b + 1) * HPC):
            acc = acc + res.results[c]["out"].astype(np.float32)
        outp[b] = acc + bo[None, :]
    return outp
